# revision 1
# baseline (speedup 1.0000x reference)
"""Trainium2 Bass kernel for nn_DSFE (dual attention + LFE depthwise-conv block).

Sharding: pure data-parallel over batch B=16 across 8 NeuronCores (2 batches
per core), no collectives. Each core runs the whole per-batch network on its
shard.

Per-batch device plan (layouts chosen so no input transpose is ever needed):
  XC   (c,n)  : strided DMA load of x (B,W,H,C) -> (C,N) per batch, n=32h+w
  QKVV (n,4C) : lhsT=XC chunks, rhs=qkvv_w^T chunks          (PE, fp32r)
  attention per head:
    qT/kT via PE transposes -> row sumsq (ACT Square+accum) -> invq/invk
    CA: S0 = q@k^T from native (n,d) chunks (PSUM accum over n-chunks);
        scaled by invq*temp (per-partition) and ones(x)invk outer (columns);
        row softmax (DVE/ACT); x_ca = exp(S)^T @ v_ca^T with 1/rowsum folded
        into the PSUM evict scale
    SA: k_proj/v_proj = k/v_sa @ E^T/F^T + bias (rank-1 ones matmul);
        A0^ = qn_T chunks @ k_proj -> (n,p) tiles; segmented softmax over
        p=16 using free-dim 0-stride broadcasts; attn^T via PE transposes;
        x_sa emitted directly in the torch "scrambled" (c'=n%512,
        n'=8d+2h+s) layout via per-(h,s,q) matmuls + stride-8 SBUF evicts
  OUTA (n,512): [x_sa@out_w^T | x_ca@out2_w^T] + (out_b|out2_b)+fc2_b,
        bias first via rank-1 ones matmul, all accumulated in one PSUM bank
  LFE: T1 = fc1 (PE, bias at ACT evict) -> 3x3 depthwise conv as 9
       shifted-AP scalar_tensor_tensor taps (6 on DVE incl. center-init,
       3 + memset on GPSIMD, two partial tiles summed) -> +dw_b, exact GELU
       (ACT) -> fc2 (PE) -> += OUTA (DVE) -> strided DMA store to y

fp32 data end-to-end; matmuls are bitcast to float32r (full-rate fp32 on the
PE) when the moving free dim is >= 256, exact fp32 otherwise.
"""

from contextlib import ExitStack

import numpy as np

import concourse.bass as bass
import concourse.mybir as mybir
import concourse.tile as tile
from concourse.masks import make_identity

FP = mybir.dt.float32
FR = mybir.dt.float32r
AF = mybir.ActivationFunctionType
ALU = mybir.AluOpType
ts = bass.ts

B, W_, H_, C = 16, 32, 32, 512
HEADS, PP, HID = 4, 16, 2048
N = H_ * W_            # 1024
D = C // HEADS         # 128
NCORES = 8
BL = B // NCORES       # 2 batches per core
NCH = N // 128         # 8
CCH = C // 128         # 4
OCH = HID // 128       # 16

NORM_EPS = 1e-12

# dev probe: when False, skip the 8 non-center conv taps (timing only)
ENABLE_TAPS = True

# depthwise 3x3 tap split across engines ((kh, kw) indices)
CENTER_TAP = (1, 1)                                  # DVE, initializes P1
DVE_TAPS = [(0, 1), (1, 0), (1, 2), (2, 1), (2, 2)]  # accumulate into P1
GPS_TAPS = [(0, 0), (0, 2), (2, 0)]                  # accumulate into P2


def _mm(nc, out, lhsT, rhs, start, stop, skip_group_check=False):
    """out = lhsT.T @ rhs ; fp32r fast path when the moving free dim >= 256."""
    if rhs.free_size() >= 256:
        lhsT = lhsT.bitcast(FR)
        rhs = rhs.bitcast(FR)
    nc.tensor.matmul(out, lhsT, rhs, start=start, stop=stop,
                     skip_group_check=skip_group_check)


def _tap_slices(kh, kw):
    """SAME-pad regions: G[r0:r1,c0:c1] += T[r0+oh:r1+oh,c0+ow:c1+ow]*w."""
    oh, ow = kh - 1, kw - 1
    r0, r1 = max(0, -oh), H_ - max(0, oh)
    c0, c1 = max(0, -ow), W_ - max(0, ow)
    return (r0, r1, c0, c1), (r0 + oh, r1 + oh, c0 + ow, c1 + ow)


def split_multi_waits(nc):
    """This environment's walrus build encodes at most ONE sync wait per
    instruction; re-host excess waits onto same-engine NoOps just before."""
    n_split = 0
    for f in nc.m.functions:
        for blk in f.blocks:
            new = []
            changed = False
            for inst in blk.instructions:
                si = inst.sync_info
                waits = list(si.on_wait) if si and si.on_wait else []
                if len(waits) > 1:
                    n_split += 1
                    changed = True
                    for w in waits[:-1]:
                        new.append(mybir.InstNoOp(
                            name=nc.get_next_instruction_name(),
                            sync_info=mybir.SyncInfo(on_wait=[w], on_update=[]),
                            bass_nofuse=True,
                            engine=inst.engine,
                        ))
                    inst.sync_info = mybir.SyncInfo(
                        on_wait=[waits[-1]],
                        on_update=list(si.on_update) if si.on_update else [],
                    )
                new.append(inst)
            if changed:
                blk.instructions = new
    return n_split


def build(split_waits=True, sim_gelu=False, loop=1, phases="AB"):
    """Build the per-core Bass module (SPMD: identical program on 8 cores).

    split_waits=False skips the walrus single-wait workaround pass (the
    injected NoOps confuse CoreSim's race detector; the pass is only needed
    for hardware codegen)."""
    nc = bass.Bass("TRN2", target_bir_lowering=False, debug=False)

    def din(name, shape):
        return nc.dram_tensor(name, list(shape), FP, kind="ExternalInput").ap()

    # DRAM scratch for device-side transposed copies of the big weights
    # (a partition-step-1 strided DMA load would degenerate to 4-byte
    #  descriptors, so we transpose once on the PE and reload contiguously)
    scratch = dict(
        qkvvT=nc.dram_tensor("qkvvT_s", [C, 4 * C], FR).ap(),
        fc1T=nc.dram_tensor("fc1T_s", [C, HID], FR).ap(),
        fc2T=nc.dram_tensor("fc2T_s", [HID, C], FR).ap(),
    )

    aps = dict(
        x=din("x", (BL, W_, H_, C)),
        qkvv_w=din("qkvv_w", (4 * C, C)),
        e_w=din("E_w", (PP, N)),
        e_b=din("E_b", (PP,)),
        f_w=din("F_w", (PP, N)),
        f_b=din("F_b", (PP,)),
        temp=din("temp", (HEADS, 1, 1)),
        temp2=din("temp2", (HEADS, 1, 1)),
        out_w=din("out_w", (C // 2, C)),
        out_b=din("out_b", (C // 2,)),
        out2_w=din("out2_w", (C // 2, C)),
        out2_b=din("out2_b", (C // 2,)),
        fc1_w=din("fc1_w", (HID, C)),
        fc1_b=din("fc1_b", (HID,)),
        dw_w=din("dw_w", (HID, 1, 3, 3)),
        dw_b=din("dw_b", (HID,)),
        fc2_w=din("fc2_w", (C, HID)),
        fc2_b=din("fc2_b", (C,)),
        y=nc.dram_tensor("y", [BL, W_, H_, C], FP, kind="ExternalOutput").ap(),
    )

    with tile.TileContext(nc) as tc:
        _emit(nc, tc, aps, scratch, sim_gelu, loop, phases)

    if split_waits:
        split_multi_waits(nc)
    return nc


def _transpose_weight(nc, tc, ident, nat_ap, dst, label):
    """PE-transpose a natural (R, K) weight into K-major form.

    dst: DRAM scratch AP of shape (K, R), or a list of K//128 SBUF tiles of
    shape (128, R) (kept resident, no DRAM round-trip)."""
    dve = nc.vector
    pe = nc.tensor
    R, K = nat_ap.shape
    RC, KC = R // 128, K // 128
    to_dram = not isinstance(dst, list)
    with ExitStack() as es:
        nat_pool = es.enter_context(tc.tile_pool(name=f"tw_nat_{label}", bufs=2))
        ps_pool = es.enter_context(
            tc.tile_pool(name=f"tw_ps_{label}", bufs=4, space="PSUM"))
        if to_dram:
            wt_pool = es.enter_context(tc.tile_pool(name=f"tw_wt_{label}", bufs=1))
            wts = [wt_pool.tile([128, R], FP, tag=f"wt{ci}", name=f"{label}wt{ci}")
                   for ci in range(KC)]
        else:
            wts = dst
        for rj in range(RC):
            nat = nat_pool.tile([128, K], FP, tag="nat", name=f"{label}nat{rj}")
            nc.sync.dma_start(nat[:], nat_ap[rj * 128:(rj + 1) * 128])
            for ci in range(KC):
                ps = ps_pool.tile([128, 128], FP, tag="tp", name=f"{label}tp")
                pe.transpose(ps[:], nat[:, ts(ci, 128)], ident[:])
                dve.tensor_copy(wts[ci][:].bitcast(FR)[:, ts(rj, 128)], ps[:])
        if to_dram:
            for ci in range(KC):
                nc.sync.dma_start(dst[ci * 128:(ci + 1) * 128],
                                  wts[ci][:].bitcast(FR))


def _emit(nc, tc, aps, scratch, sim_gelu=False, loop=1, phases="AB"):
    dve = nc.vector
    pe = nc.tensor

    with ExitStack() as es:
        const = es.enter_context(tc.tile_pool(name="const", bufs=1))

        ident = const.tile([128, 128], FP)
        make_identity(nc, ident)
        ones_row = const.tile([1, 128], FP)
        dve.memset(ones_row[:], 1.0)

        # one-time device-side weight transposes into DRAM scratch
        _transpose_weight(nc, tc, ident, aps["qkvv_w"], scratch["qkvvT"], "qkvv")
        _transpose_weight(nc, tc, ident, aps["fc1_w"], scratch["fc1T"], "fc1")
        _transpose_weight(nc, tc, ident, aps["fc2_w"], scratch["fc2T"], "fc2")

        # temp/temp2 broadcast to all 128 partitions: (128, HEADS)
        tcol = const.tile([128, HEADS], FP)
        nc.sync.dma_start(
            tcol[:], aps["temp"].rearrange("h a b -> (a b) h").to_broadcast((128, HEADS)))
        t2col = const.tile([128, HEADS], FP)
        nc.sync.dma_start(
            t2col[:], aps["temp2"].rearrange("h a b -> (a b) h").to_broadcast((128, HEADS)))

        # E^T / F^T as (n-part r, n-chunk k, p)
        ewt = const.tile([128, NCH, PP], FP)
        fwt = const.tile([128, NCH, PP], FP)
        with ExitStack() as ef:
            ef_pool = ef.enter_context(tc.tile_pool(name="ef_nat", bufs=2))
            ef_ps = ef.enter_context(tc.tile_pool(name="ef_ps", bufs=4, space="PSUM"))
            e_nat = ef_pool.tile([PP, N], FP, tag="nat", name="e_nat")
            nc.sync.dma_start(e_nat[:], aps["e_w"][:])
            f_nat = ef_pool.tile([PP, N], FP, tag="nat", name="f_nat")
            nc.sync.dma_start(f_nat[:], aps["f_w"][:])
            for k in range(NCH):
                ps = ef_ps.tile([128, PP], FP, tag="tp", name="e_tp")
                pe.transpose(ps[:], e_nat[:, ts(k, 128)], ident[0:PP, 0:PP])
                dve.tensor_copy(ewt[:].bitcast(FR)[:, k, :], ps[:])
                ps2 = ef_ps.tile([128, PP], FP, tag="tp", name="f_tp")
                pe.transpose(ps2[:], f_nat[:, ts(k, 128)], ident[0:PP, 0:PP])
                dve.tensor_copy(fwt[:].bitcast(FR)[:, k, :], ps2[:])
        eb_row = const.tile([1, PP], FP)
        nc.sync.dma_start(eb_row[:], aps["e_b"].unsqueeze(0))
        fb_row = const.tile([1, PP], FP)
        nc.sync.dma_start(fb_row[:], aps["f_b"].unsqueeze(0))

        # free-dim bias row for OUTA: concat(out_b, out2_b) + fc2_b
        br_cat = const.tile([1, C], FP)
        nc.sync.dma_start(br_cat[:, 0:C // 2], aps["out_b"].unsqueeze(0))
        nc.sync.dma_start(br_cat[:, C // 2:C], aps["out2_b"].unsqueeze(0))
        fcb_row = const.tile([1, C], FP)
        nc.sync.dma_start(fcb_row[:], aps["fc2_b"].unsqueeze(0))
        brf = const.tile([1, C], FP)
        dve.tensor_add(brf[:].bitcast(FR), br_cat[:], fcb_row[:])
        ones_fr = const.tile([1, 128], FP)
        nc.scalar.activation(ones_fr[:].bitcast(FR), ones_row[:], AF.Identity)

        # per-partition (o) bias/weight columns for the LFE branch
        fb1 = const.tile([128, OCH], FP)
        nc.sync.dma_start(fb1[:], aps["fc1_b"].rearrange("(k r) -> r k", r=128))
        dwb = const.tile([128, OCH], FP)
        nc.sync.dma_start(dwb[:], aps["dw_b"].rearrange("(k r) -> r k", r=128))
        dww = const.tile([128, OCH * 9], FP)
        for oc in range(OCH):
            nc.sync.dma_start(
                dww[:, oc * 9:(oc + 1) * 9],
                aps["dw_w"][oc * 128:(oc + 1) * 128].rearrange("r a kh kw -> r (a kh kw)"))

        ndww = const.tile([128, OCH * 9], FP)
        dve.tensor_scalar_mul(ndww[:], dww[:], -1.0)


        # out_w^T / out2_w^T chunks (c-part, j), transposed on-chip, resident
        owt = [const.tile([128, C // 2], FP, tag=f"owt{q}", name=f"owt{q}")
               for q in range(CCH)]
        owt2 = [const.tile([128, C // 2], FP, tag=f"owt2{q}", name=f"owt2{q}")
                for q in range(CCH)]
        _transpose_weight(nc, tc, ident, aps["out_w"], owt, "outw")
        _transpose_weight(nc, tc, ident, aps["out2_w"], owt2, "outw2")

        cst = dict(ident=ident, ones_row=ones_row, ones_fr=ones_fr,
                   ndww=ndww, tcol=tcol, t2col=t2col,
                   ewt=ewt, fwt=fwt, eb_row=eb_row, fb_row=fb_row, brf=brf,
                   fb1=fb1, dwb=dwb, dww=dww, owt=owt, owt2=owt2)

        for rep in range(loop):
            for b in range(BL):
                _emit_batch(nc, tc, b, aps, cst, scratch, sim_gelu,
                            label=f"{rep}_{b}", phases=phases)


def _emit_batch(nc, tc, b, aps, cst, scratch, sim_gelu=False, label=None,
                phases="AB"):
    if label is None:
        label = str(b)
    act = nc.scalar
    dve = nc.vector
    gps = nc.gpsimd
    pe = nc.tensor
    ident = cst["ident"]
    ones_row = cst["ones_row"]

    with ExitStack() as bs:
        xc_pool = bs.enter_context(tc.tile_pool(name=f"xc{label}", bufs=1))
        outa_pool = bs.enter_context(tc.tile_pool(name=f"outa{label}", bufs=1))

        # ---- XC (c, n), n = 32h + w: load x[b] rows naturally (m = 32w + h
        # order, contiguous 2KB runs), PE-transpose 128x128 blocks, and fix
        # the h/w swap with a permuted SBUF write AP on the evict.
        xflat = aps["x"][b].rearrange("w h c -> (w h) c")  # (N, C), m-order
        xc = [xc_pool.tile([128, N], FP, tag=f"xc{i}", name=f"xc{i}")
              for i in range(CCH)]
        with ExitStack() as xl:
            xn_pool = xl.enter_context(tc.tile_pool(name=f"xn{label}", bufs=3))
            xn_ps = xl.enter_context(
                tc.tile_pool(name=f"xnps{label}", bufs=4, space="PSUM"))
            for kk in range(NCH):
                xn = xn_pool.tile([128, C], FP, tag="xn", name="xn")
                nc.sync.dma_start(xn[:], xflat[kk * 128:(kk + 1) * 128])
                for i in range(CCH):
                    ps = xn_ps.tile([128, 128], FP, tag="xt", name="xt")
                    pe.transpose(ps[:], xn[:, ts(i, 128)], ident[:])
                    xcv = xc[i][:].bitcast(FR).rearrange("p (h w) -> p h w", w=W_)
                    dve.tensor_copy(
                        xcv[:, :, 4 * kk:4 * kk + 4].transpose([0, 2, 1]),
                        ps[:].rearrange("p (a b) -> p a b", b=32))

        outa = [outa_pool.tile([128, C], FP, tag=f"outa{m}", name=f"outa{m}") for m in range(NCH)]
        if phases != "AB":
            # timing-only partial builds: keep outa written+read so Tile's
            # allocator is happy, and always touch y
            for m in range(NCH):
                nc.vector.memset(outa[m][:], 0.0)
        if "B" not in phases:
            ydst0 = aps["y"][b].rearrange("w h c -> h w c")
            nc.sync.dma_start(ydst0[0], outa[0][0:32, :])

        # ======================= phase A: attention =======================
        if "A" in phases:
          with ExitStack() as pa:
            qkvv_pool = pa.enter_context(tc.tile_pool(name=f"qkvv{label}", bufs=1))
            ps_big = pa.enter_context(tc.tile_pool(name=f"psb{label}", bufs=2, space="PSUM"))
            ps_med = pa.enter_context(tc.tile_pool(name=f"psm{label}", bufs=4, space="PSUM"))

            # ---- QKVV (n, 4C)
            qkvv = [qkvv_pool.tile([128, 4 * C], FP, tag=f"qkvv{k}", name=f"qkvv{k}") for k in range(NCH)]
            with ExitStack() as pw:
                qw_pool = pw.enter_context(tc.tile_pool(name=f"qw{label}", bufs=1))
                qw = []
                for i in range(CCH):
                    t = qw_pool.tile([128, 4 * C], FP, tag=f"qw{i}")
                    nc.sync.dma_start(
                        t[:].bitcast(FR), scratch["qkvvT"][i * 128:(i + 1) * 128])
                    qw.append(t)
                for k in range(NCH):
                    for half in range(2):
                        ps = ps_big.tile([128, 1024], FP, tag="big")
                        for j in range(2):
                            for i in range(CCH):
                                _mm(nc, ps[:, ts(j, 512)],
                                    xc[i][:, ts(k, 128)],
                                    qw[i][:, half * 1024 + j * 512:half * 1024 + (j + 1) * 512],
                                    start=(i == 0), stop=(i == CCH - 1))
                        dve.tensor_copy(qkvv[k][:].bitcast(FR)[:, ts(half, 1024)], ps[:])

            with ExitStack() as ph:
                at_pool = pa.enter_context(tc.tile_pool(name=f"at{label}", bufs=2))
                xca_pool = pa.enter_context(tc.tile_pool(name=f"xca{label}", bufs=1))
                xsa_pool = pa.enter_context(tc.tile_pool(name=f"xsa{label}", bufs=1))

                xsa = [xsa_pool.tile([128, N], FP, tag=f"xsa{q}", name=f"xsa{q}") for q in range(CCH)]
                xca = []

                for h in range(HEADS):
                    qc = h * 128            # q columns in QKVV
                    kc = C + h * 128        # k columns
                    vc = 2 * C + h * 128    # v_ca columns
                    sc = 3 * C + h * 128    # v_sa columns

                    # ---- q^T (d, n) + row norms
                    qT_ps = ps_big.tile([128, N], FP, tag="big")
                    for k in range(NCH):
                        pe.transpose(qT_ps[:, ts(k, 128)],
                                     qkvv[k][:, qc:qc + 128], ident[:])
                    sq = at_pool.tile([128, N], FP, tag="sq")
                    ssq = at_pool.tile([128, 1], FP, tag="ssq")
                    act.activation(sq[:], qT_ps[:], AF.Square, accum_out=ssq[:])
                    nrmq = at_pool.tile([128, 1], FP, tag="nrmq")
                    act.activation(nrmq[:], ssq[:], AF.Sqrt)
                    dve.tensor_scalar_max(nrmq[:], nrmq[:], NORM_EPS)
                    invq = at_pool.tile([128, 1], FP, tag="invq")
                    dve.reciprocal(invq[:], nrmq[:])
                    invq_t = at_pool.tile([128, 1], FP, tag="invq_t")
                    dve.tensor_mul(invq_t[:], invq[:], cst["tcol"][:, h:h + 1])
                    qn_t = at_pool.tile([128, N], FP, tag="qn_t")
                    act.activation(qn_t[:], qT_ps[:], AF.Identity, scale=invq[:])

                    # ---- k^T -> invk (column + row + ones(x)invk outer)
                    kT_ps = ps_big.tile([128, N], FP, tag="big")
                    for k in range(NCH):
                        pe.transpose(kT_ps[:, ts(k, 128)],
                                     qkvv[k][:, kc:kc + 128], ident[:])
                    sqk = at_pool.tile([128, N], FP, tag="sq")
                    ssqk = at_pool.tile([128, 1], FP, tag="ssqk")
                    act.activation(sqk[:], kT_ps[:], AF.Square, accum_out=ssqk[:])
                    nrmk = at_pool.tile([128, 1], FP, tag="nrmk")
                    act.activation(nrmk[:], ssqk[:], AF.Sqrt)
                    dve.tensor_scalar_max(nrmk[:], nrmk[:], NORM_EPS)
                    invk = at_pool.tile([128, 1], FP, tag="invk")
                    dve.reciprocal(invk[:], nrmk[:])
                    ikr_ps = ps_med.tile([1, 128], FP, tag="med")
                    pe.transpose(ikr_ps[:], invk[:], ident[:])
                    ikr = at_pool.tile([1, 128], FP, tag="ikr")
                    dve.tensor_copy(ikr[:], ikr_ps[:])
                    colsc_ps = ps_med.tile([128, 128], FP, tag="med")
                    nc.tensor.matmul(colsc_ps[:], ones_row[:], ikr[:],
                                     start=True, stop=True)
                    colsc = at_pool.tile([128, 128], FP, tag="colsc")
                    dve.tensor_copy(colsc[:], colsc_ps[:])

                    # ---- CA scores S0 = q @ k^T (use 256-wide rhs for fp32r)
                    s_ps = ps_med.tile([128, 256], FP, tag="med")
                    for k in range(NCH):
                        _mm(nc, s_ps[:], qkvv[k][:, qc:qc + 128],
                            qkvv[k][:, kc:kc + 256],
                            start=(k == 0), stop=(k == NCH - 1))
                    s_sb = at_pool.tile([128, 128], FP, tag="s_sb")
                    dve.scalar_tensor_tensor(s_sb[:], s_ps[:, 0:128], invq_t[:],
                                             colsc[:], op0=ALU.mult, op1=ALU.mult)

                    # ---- CA row softmax (1/sum folded into x_ca evict)
                    negmax = at_pool.tile([128, 1], FP, tag="negmax")
                    dve.tensor_reduce(negmax[:], s_sb[:], axis=mybir.AxisListType.X,
                                      op=ALU.max, negate=True)
                    e_sb = at_pool.tile([128, 128], FP, tag="e_sb")
                    sume = at_pool.tile([128, 1], FP, tag="sume")
                    act.activation(e_sb[:], s_sb[:], AF.Exp, bias=negmax[:],
                                   accum_out=sume[:])
                    rex = at_pool.tile([128, 1], FP, tag="rex")
                    dve.reciprocal(rex[:], sume[:])

                    et_ps = ps_med.tile([128, 128], FP, tag="med")
                    pe.transpose(et_ps[:], e_sb[:], ident[:])
                    et_sb = at_pool.tile([128, 128], FP, tag="et_sb")
                    dve.tensor_copy(et_sb[:].bitcast(FR), et_ps[:])

                    # ---- v_ca^T (e, n)
                    vt_ps = ps_big.tile([128, N], FP, tag="big")
                    for k in range(NCH):
                        pe.transpose(vt_ps[:, ts(k, 128)],
                                     qkvv[k][:, vc:vc + 128], ident[:])
                    vt_sb = at_pool.tile([128, N], FP, tag="vt_sb")
                    dve.tensor_copy(vt_sb[:].bitcast(FR), vt_ps[:])

                    # ---- x_ca (d, n) = (1/sum) * E^T.T @ v_ca^T
                    xca_ps = ps_big.tile([128, N], FP, tag="big")
                    for j in range(2):
                        _mm(nc, xca_ps[:, ts(j, 512)], et_sb[:],
                            vt_sb[:, ts(j, 512)], start=True, stop=True)
                    xca_h = xca_pool.tile([128, N], FP, tag=f"xca{h}")
                    act.activation(xca_h[:].bitcast(FR), xca_ps[:], AF.Identity,
                                   scale=rex[:])
                    xca.append(xca_h)

                    # ---- SA: k_proj / v_proj (d, p), bias first
                    kp_ps = ps_med.tile([128, PP], FP, tag="med")
                    nc.tensor.matmul(kp_ps[:], ones_row[:], cst["eb_row"][:],
                                     start=True, stop=False)
                    for k in range(NCH):
                        _mm(nc, kp_ps[:], qkvv[k][:, kc:kc + 128],
                            cst["ewt"][:, k, :], start=False, stop=(k == NCH - 1))
                    kp_sb = at_pool.tile([128, PP], FP, tag="kp_sb")
                    dve.tensor_copy(kp_sb[:], kp_ps[:])

                    vp_ps = ps_med.tile([128, PP], FP, tag="med")
                    nc.tensor.matmul(vp_ps[:], ones_row[:], cst["fb_row"][:],
                                     start=True, stop=False)
                    for k in range(NCH):
                        _mm(nc, vp_ps[:], qkvv[k][:, sc:sc + 128],
                            cst["fwt"][:, k, :], start=False, stop=(k == NCH - 1))
                    vp_sb = at_pool.tile([128, PP], FP, tag="vp_sb")
                    dve.tensor_copy(vp_sb[:], vp_ps[:])
                    vpt_ps = ps_med.tile([16, 128], FP, tag="med")
                    pe.transpose(vpt_ps[:], vp_sb[:], ident[:])
                    vpt_sb = at_pool.tile([16, 128], FP, tag="vpt_sb")
                    dve.tensor_copy(vpt_sb[:], vpt_ps[:])

                    # ---- A0 (n, p) per n-chunk, all 8 in one (128, 8, 16)
                    a_ps = ps_med.tile([128, 128], FP, tag="med")
                    a3 = a_ps[:].rearrange("p (k s) -> p k s", s=PP)
                    for k in range(NCH):
                        nc.tensor.matmul(a3[:, k, :], qn_t[:, ts(k, 128)],
                                         kp_sb[:], start=True, stop=True)

                    # ---- segmented softmax over p (free-dim broadcasts)
                    amax = at_pool.tile([128, NCH], FP, tag="amax")
                    dve.tensor_reduce(amax[:], a3, axis=mybir.AxisListType.X,
                                      op=ALU.max)
                    zt = at_pool.tile([128, 128], FP, tag="zt")
                    zt3 = zt[:].rearrange("p (k s) -> p k s", s=PP)
                    dve.tensor_sub(zt3, a3,
                                   amax[:].unsqueeze(2).to_broadcast((128, NCH, PP)))
                    ez = at_pool.tile([128, 128], FP, tag="ez")
                    act.activation(ez[:], zt[:], AF.Exp, scale=cst["t2col"][:, h:h + 1])
                    ez3 = ez[:].rearrange("p (k s) -> p k s", s=PP)
                    esum = at_pool.tile([128, NCH], FP, tag="esum")
                    dve.tensor_reduce(esum[:], ez3, axis=mybir.AxisListType.X,
                                      op=ALU.add)
                    rsum = at_pool.tile([128, NCH], FP, tag="rsum")
                    dve.reciprocal(rsum[:], esum[:])
                    attn = at_pool.tile([128, 128], FP, tag="attn")
                    attn3 = attn[:].rearrange("p (k s) -> p k s", s=PP)
                    dve.tensor_mul(attn3, ez3,
                                   rsum[:].unsqueeze(2).to_broadcast((128, NCH, PP)))

                    # ---- attn^T (p, n)
                    at_ps = ps_big.tile([16, N], FP, tag="big")
                    for k in range(NCH):
                        pe.transpose(at_ps[:, ts(k, 128)], attn3[:, k, :], ident[:])
                    at_sb = at_pool.tile([16, N], FP, tag="at_sb")
                    dve.tensor_copy(at_sb[:], at_ps[:])

                    # ---- x_sa in scrambled (c'=n%512, n'=8d+2h+s) layout
                    for s in range(2):
                        for q in range(CCH):
                            k = 4 * s + q
                            xs_ps = ps_med.tile([128, 128], FP, tag="med")
                            nc.tensor.matmul(xs_ps[:], at_sb[:, ts(k, 128)],
                                             vpt_sb[:], start=True, stop=True)
                            dst = xsa[q][:].bitcast(FR).rearrange("p (d e) -> p d e", e=8)[:, :, 2 * h + s]
                            dve.tensor_copy(dst, xs_ps[:])

                # ---- OUTA (n, 512) = bias + [x_sa@out_w^T | x_ca@out2_w^T]
                for m in range(NCH):
                    o_ps = ps_big.tile([128, C], FP, tag="big")
                    _mm(nc, o_ps[:], cst["ones_fr"][:], cst["brf"][:],
                        start=True, stop=False, skip_group_check=True)
                    for q in range(CCH):
                        _mm(nc, o_ps[:, 0:C // 2], xsa[q][:, ts(m, 128)],
                            cst["owt"][q][:], start=False, stop=(q == CCH - 1),
                            skip_group_check=True)
                    for h in range(HEADS):
                        _mm(nc, o_ps[:, C // 2:C], xca[h][:, ts(m, 128)],
                            cst["owt2"][h][:], start=False, stop=(h == HEADS - 1),
                            skip_group_check=True)
                    dve.tensor_copy(outa[m][:], o_ps[:, 0:C])

        # ======================= phase B: LFE =======================
        if "B" in phases:
          with ExitStack() as pb:
            g_pool = pb.enter_context(tc.tile_pool(name=f"g{label}", bufs=1))
            ps_big2 = pb.enter_context(tc.tile_pool(name=f"psb2{label}", bufs=4, space="PSUM"))
            g_tiles = []

            # fc2 weights early so their DMAs overlap the conv phase
            fc2t_pool = pb.enter_context(tc.tile_pool(name=f"fc2t{label}", bufs=1))
            fc2t = []
            for oc in range(OCH):
                t = fc2t_pool.tile([128, C], FP, tag=f"fc2t{oc}")
                nc.sync.dma_start(
                    t[:].bitcast(FR), scratch["fc2T"][oc * 128:(oc + 1) * 128])
                fc2t.append(t)

            with ExitStack() as pf1:
                fc1t_pool = pf1.enter_context(tc.tile_pool(name=f"fc1t{label}", bufs=1))
                conv_pool = pf1.enter_context(tc.tile_pool(name=f"conv{label}", bufs=2))
                fc1t = []
                for i in range(CCH):
                    t = fc1t_pool.tile([128, HID], FP, tag=f"fc1t{i}")
                    nc.sync.dma_start(
                        t[:].bitcast(FR), scratch["fc1T"][i * 128:(i + 1) * 128])
                    fc1t.append(t)

                taps = [CENTER_TAP] + DVE_TAPS + GPS_TAPS
                for oc in range(OCH):
                    # diag(w_tap) built on the fly: ident * broadcast(w)
                    dg = conv_pool.tile([128, 9 * 128], FP, tag="dg", name="dg")
                    dve.tensor_mul(
                        dg[:].bitcast(FR).rearrange("p (t j) -> p t j", j=128),
                        cst["ident"][:].unsqueeze(1).to_broadcast((128, 9, 128)),
                        cst["dww"][:, oc * 9:(oc + 1) * 9].unsqueeze(2)
                            .to_broadcast((128, 9, 128)))
                    t1_ps = ps_big2.tile([128, N], FP, tag="big")
                    for j in range(2):
                        for i in range(CCH):
                            _mm(nc, t1_ps[:, ts(j, 512)],
                                fc1t[i][:, ts(oc, 128)], xc[i][:, ts(j, 512)],
                                start=(i == 0), stop=(i == CCH - 1))
                    # T lives in a zero-padded tile (32 pad elements both
                    # sides) so every shifted tap reads a full in-bounds range
                    t_sb = conv_pool.tile([128, N + 96], FP, tag="t_sb")
                    dve.memset(t_sb[:, 0:48], 0.0)
                    dve.memset(t_sb[:, N + 48:N + 96], 0.0)
                    act.activation(t_sb[:].bitcast(FR)[:, 48:N + 48], t1_ps[:],
                                   AF.Identity, bias=cst["fb1"][:, oc:oc + 1])
                    tflat = t_sb[:].bitcast(FR)
                    tvv = t_sb[:, 48:N + 48].rearrange("p (h w) -> p h w", w=W_)
                    dgf = dg[:].bitcast(FR)

                    # 3x3 depthwise conv as 9 diag(w_tap) matmuls on flat
                    # row-major APs (shift s = 32*oh + ow), accumulating in
                    # PSUM; the w-wrap column of each ow!=0 tap is fixed up
                    # afterwards with a small negated-weight DVE op.
                    conv_ps = ps_big2.tile([128, N], FP, tag="big", name="conv_ps")
                    cpv = conv_ps[:].rearrange("p (h w) -> p h w", w=W_)
                    for idx, (kh, kw) in enumerate(taps):
                        t_i = 3 * kh + kw
                        oh, ow = kh - 1, kw - 1
                        s = 32 * oh + ow
                        for u in (0, 512):
                            nc.tensor.matmul(
                                conv_ps[:, u:u + 512],
                                dgf[:, ts(t_i, 128)],
                                tflat[:, 48 + u + s:48 + u + s + 512],
                                start=(idx == 0), stop=(idx == len(taps) - 1),
                                skip_group_check=True)
                    for kh, kw in taps:
                        t_i = 3 * kh + kw
                        oh, ow = kh - 1, kw - 1
                        if ow == 0:
                            continue
                        s = 32 * oh + ow
                        A, Bn = max(0, -s), N - max(0, s)
                        wb = 31 if ow == 1 else 0
                        h0 = -((A - wb) // -32)          # ceil div
                        h1 = (Bn - 1 - wb) // 32 + 1
                        nwc = cst["ndww"][:, oc * 9 + t_i:oc * 9 + t_i + 1]
                        ih0 = h0 + oh + (1 if ow == 1 else -1)
                        iw = 0 if ow == 1 else 31
                        dve.scalar_tensor_tensor(
                            cpv[:, h0:h1, wb:wb + 1],
                            tvv[:, ih0:ih0 + (h1 - h0), iw:iw + 1], nwc,
                            cpv[:, h0:h1, wb:wb + 1],
                            op0=ALU.mult, op1=ALU.add)

                    g_sb = g_pool.tile([128, N], FP, tag=f"g{oc}")
                    if sim_gelu:
                        # CoreSim has no Gelu/Erf; x*sigmoid(1.702x) is
                        # numerically indistinguishable at our value scale
                        gt = conv_pool.tile([128, N], FP, tag="gt", bufs=1)
                        act.activation(gt[:], conv_ps[:], AF.Identity,
                                       bias=cst["dwb"][:, oc:oc + 1])
                        sg = conv_pool.tile([128, N], FP, tag="sg", bufs=1)
                        act.activation(sg[:], gt[:], AF.Sigmoid, scale=1.702)
                        dve.tensor_mul(g_sb[:].bitcast(FR), gt[:], sg[:])
                    else:
                        act.activation(g_sb[:].bitcast(FR), conv_ps[:], AF.Gelu,
                                       bias=cst["dwb"][:, oc:oc + 1])
                    g_tiles.append(g_sb)

            # ---- fc2 + OUTA -> y
            with ExitStack() as pf2:
                fin_pool = pf2.enter_context(tc.tile_pool(name=f"fin{label}", bufs=2))

                ydst = aps["y"][b].rearrange("w h c -> h w c")  # (H, W, C)
                for m in range(NCH):
                    f_ps = ps_big2.tile([128, C], FP, tag="big")
                    for oc in range(OCH):
                        _mm(nc, f_ps[:], g_tiles[oc][:, ts(m, 128)], fc2t[oc][:],
                            start=(oc == 0), stop=(oc == OCH - 1))
                    fin = fin_pool.tile([128, C], FP, tag="fin")
                    dve.tensor_add(fin[:], f_ps[:], outa[m][:])
                    for g in range(4):
                        h_row = 4 * m + g
                        nc.sync.dma_start(ydst[h_row], fin[32 * g:32 * (g + 1), :])


_BUILD_CACHE = {}


def _get_nc():
    if "nc" not in _BUILD_CACHE:
        _BUILD_CACHE["nc"] = build()
    return _BUILD_CACHE["nc"]


def kernel(**inputs):
    from concourse.bass_utils import run_bass_kernel_spmd

    def f32(a):
        return np.ascontiguousarray(np.asarray(a, dtype=np.float32))

    x = f32(inputs["x"])
    assert x.shape == (B, W_, H_, C), x.shape
    common = {k: f32(inputs[k]) for k in
              ("qkvv_w", "E_w", "E_b", "F_w", "F_b", "temp", "temp2",
               "out_w", "out_b", "out2_w", "out2_b",
               "fc1_w", "fc1_b", "dw_w", "dw_b", "fc2_w", "fc2_b")}

    nc = _get_nc()
    in_maps = []
    for c in range(NCORES):
        m = dict(common)
        m["x"] = np.ascontiguousarray(x[c * BL:(c + 1) * BL])
        in_maps.append(m)

    res = run_bass_kernel_spmd(nc, in_maps, list(range(NCORES)))
    out = np.concatenate([res.results[c]["y"] for c in range(NCORES)], axis=0)
    return out.astype(np.float32)

